# revision 8
# baseline (speedup 1.0000x reference)
"""Causal self-attention (B=4, T=2048, C=1024, H=16) on 8 trn2 NeuronCores.

Sharding: tensor-parallel over heads for QKV projection + attention
(2 heads/core), then an on-device AllToAll reshards from head-sharded to
row-sharded so each core computes the output projection (full C
contraction) for its 1024 rows. Host gather is pure concatenation.

Layout trick: attention is computed in "transposed" orientation
S^T[k, q] = (K Q^T), so softmax's reduction lands on the PSUM
accumulation path: V is augmented with a ones column, making the PV
matmul produce both y^T (rows 0..63) and the softmax denominator
(row 64) in one accumulation. No max-subtraction is needed (logits are
small: weights scaled by 0.02), and no P-transpose is needed anywhere.
"""

import sys

for _p in ("/opt/trn_rl_repo",):
    if _p not in sys.path:
        sys.path.insert(0, _p)

import numpy as np
import ml_dtypes

B, T, C, H, HS = 4, 2048, 1024, 16, 64
NCORES = 8
HPC = H // NCORES            # heads per core = 2
CPC = HPC * HS               # channels per core = 128
ROWS = B * T                 # 8192
RPC = ROWS // NCORES         # rows per core = 1024
NKT = T // 128               # k-tiles per batch = 16
NQC = T // 512               # q-chunks per batch = 4

BF16 = ml_dtypes.bfloat16

_CACHE: dict = {}


def _apply_tile_tail_patch(tile_mod):
    """This container's walrus rejects CTRL-class instructions (Drain/NoOp)
    carrying semaphore waits. Re-emit TileContext's tail waits as individual
    EventSemaphore waits and use the sem-only barrier variant."""
    import bass_rust
    from concourse.vector_clock import ScopedClock

    if getattr(tile_mod.TileContext, "_tail_patch_applied", False):
        return

    def _drain_and_barrier(self, tick_clock, wait_clock):
        collector = self.nc.sync.nop(nofuse=True, hint="tile_tail_wait")
        wait_clock.add_sem_waits(
            collector.ins, ScopedClock({None: tick_clock.global_clock})
        )
        si = collector.ins.sync_info
        waits = list(si.on_wait) if si is not None else []
        collector.ins.sync_info = None
        for w in waits:
            assert w.wait_mode == "sem-ge-imm", w
            self.nc.sync.wait_ge(
                bass_rust.SemaphoreHandle(w.ant_name, w.id), w.wait_value
            )

        self.nc.all_engine_barrier(sem_only=True)
        assert self.sems is not None
        popped = self.nc._tile_sem_poison_stack.pop()
        assert popped is self._sem_poison
        self.nc.clear_and_free_semaphores(list(self.sems.allocated().values()))
        self.nc.all_engine_barrier(sem_only=True)

    tile_mod.TileContext._drain_and_barrier = _drain_and_barrier
    tile_mod.TileContext._tail_patch_applied = True


def _build():
    import concourse.bass as bass
    import concourse.bacc as bacc
    import concourse.mybir as mybir
    import concourse.tile as tile

    dt = mybir.dt
    F32 = dt.float32
    BF = dt.bfloat16
    Exp = mybir.ActivationFunctionType.Exp
    Ident = mybir.ActivationFunctionType.Identity

    # Bacc (not plain Bass): its compile pipeline runs
    # generate_event_semaphores, which splits multi-wait sync_info into
    # EventSemaphore instructions — the walrus here accepts at most one
    # wait per instruction. It also inserts gpsimd library loads and
    # activation-table loads.
    nc = bacc.Bacc(num_devices=NCORES)

    # Inputs (per-core unless noted). xT is x transposed: [C, B*T].
    xT = nc.dram_tensor("xT", [C, ROWS], BF, kind="ExternalInput")
    wqk = nc.dram_tensor("wqk", [C, 2 * CPC], BF, kind="ExternalInput")
    wv = nc.dram_tensor("wv", [C, CPC], BF, kind="ExternalInput")
    bq = nc.dram_tensor("bq", [CPC, 1], F32, kind="ExternalInput")   # prescaled by 1/8
    bk = nc.dram_tensor("bk", [CPC, 1], F32, kind="ExternalInput")
    wp = nc.dram_tensor("wp", [C, C], BF, kind="ExternalInput")      # full c_proj_w
    bprime = nc.dram_tensor("bprime", [1, C], BF, kind="ExternalInput")
    maskd = nc.dram_tensor("maskd", [128, 128], BF, kind="ExternalInput")
    out = nc.dram_tensor("out", [RPC, C], F32, kind="ExternalOutput")

    with tile.TileContext(nc) as tc:
        with (
            tc.tile_pool(name="const", bufs=1) as constp,
            tc.tile_pool(name="big", bufs=1) as bigp,
            tc.tile_pool(name="xin", bufs=3) as xinp,
            tc.tile_pool(name="work", bufs=3) as workp,
            tc.tile_pool(name="ps", bufs=7, space="PSUM") as psp,
            tc.tile_pool(name="dram", bufs=1, space="DRAM") as dramp,
        ):
            # ---- constants ----
            wqk_sb = constp.tile([128, 8, 2 * CPC], BF, tag="wqk")
            nc.sync.dma_start(wqk_sb[:], wqk.rearrange("(ct p) o -> p ct o", p=128))
            wv_sb = constp.tile([128, 8, CPC], BF, tag="wv")
            nc.sync.dma_start(wv_sb[:], wv.rearrange("(ct p) o -> p ct o", p=128))
            wp_sb = constp.tile([128, 8, C], BF, tag="wp")
            nc.sync.dma_start(wp_sb[:], wp.rearrange("(ct p) o -> p ct o", p=128))
            bq_sb = constp.tile([CPC, 1], F32, tag="bq")
            nc.sync.dma_start(bq_sb[:], bq[:])
            bk_sb = constp.tile([CPC, 1], F32, tag="bk")
            nc.sync.dma_start(bk_sb[:], bk[:])
            bprime_sb = constp.tile([1, C], BF, tag="bprime")
            nc.sync.dma_start(bprime_sb[:], bprime[:])
            mask_sb = constp.tile([128, 128], BF, tag="mask")
            nc.sync.dma_start(mask_sb[:], maskd[:])
            ones_sb = constp.tile([1, 128], BF, tag="ones")
            nc.vector.memset(ones_sb[:], 1.0)
            onesf_sb = constp.tile([1, 64], F32, tag="onesf")
            nc.vector.memset(onesf_sb[:], 1.0)

            # ---- persistent intermediates ----
            qT_sb = bigp.tile([CPC, ROWS], BF, tag="qT")     # [2*64 ch, 8192]
            kT_sb = bigp.tile([CPC, ROWS], BF, tag="kT")
            # v' per global k-tile: [128 rows, 64 slots, 2 heads * 65] with a
            # ones column at slot offsets 64 and 129.
            vp_sb = bigp.tile([128, NKT * B, 2 * 65], BF, tag="vp")
            nc.vector.memset(vp_sb[:, :, 64:65], 1.0)
            nc.vector.memset(vp_sb[:, :, 129:130], 1.0)

            a2a_in = dramp.tile([NCORES * CPC, RPC], BF)
            a2a_out = dramp.tile([NCORES * CPC, RPC], BF)

            # ================= Phase 1: QKV projection =================
            # qT/kT: out[oc, row] = sum_c w[c, oc] * xT[c, row]
            # v:     out[row, oc] = sum_c xT[c, row] * wv[c, oc]
            xT_r = xT.rearrange("(ct p) r -> p ct r", p=128)
            for r in range(16):  # 512-row chunks
                rs = slice(r * 512, (r + 1) * 512)
                xt = xinp.tile([128, 8, 512], BF, tag="xt")
                nc.sync.dma_start(xt[:], xT_r[:, :, rs])

                q_ps = psp.tile([128, 512], F32, tag="ps", name=f"qps_{r}")
                k_ps = psp.tile([128, 512], F32, tag="ps", name=f"kps_{r}")
                # one PSUM tile (= one bank) per accumulation group: start=True
                # clears the whole bank, so groups must not share one
                v_pst = [
                    psp.tile([128, 128], F32, tag="ps", name=f"vps_{r}_{t}")
                    for t in range(4)
                ]
                for ct in range(8):
                    st, sp = (ct == 0), (ct == 7)
                    nc.tensor.matmul(
                        q_ps[:], wqk_sb[:, ct, 0:CPC], xt[:, ct, :], start=st, stop=sp
                    )
                    nc.tensor.matmul(
                        k_ps[:], wqk_sb[:, ct, CPC:], xt[:, ct, :], start=st, stop=sp
                    )
                    for t in range(4):
                        nc.tensor.matmul(
                            v_pst[t][:],
                            xt[:, ct, t * 128 : (t + 1) * 128],
                            wv_sb[:, ct, :],
                            start=st,
                            stop=sp,
                        )
                # copy-out with bias (per-partition) and 1/8 scale folded into q
                nc.scalar.activation(qT_sb[:, rs], q_ps[:], Ident, bias=bq_sb[:], scale=0.125)
                nc.scalar.activation(kT_sb[:, rs], k_ps[:], Ident, bias=bk_sb[:], scale=1.0)
                for t in range(4):
                    slot = 4 * r + t
                    nc.vector.tensor_copy(
                        out=vp_sb[:, slot, 0:64], in_=v_pst[t][:, 0:64]
                    )
                    nc.vector.tensor_copy(
                        out=vp_sb[:, slot, 65:129], in_=v_pst[t][:, 64:128]
                    )

            # ================= Phase 2: attention =================
            for b in range(B):
                for h in range(HPC):
                    hp = slice(h * 64, (h + 1) * 64)        # partition slice for this head
                    vc = slice(h * 65, (h + 1) * 65)        # v' column slice
                    for qc in range(NQC):                    # 512-wide q chunks
                        q0 = qc * 512
                        grow = b * T + q0                    # global row of chunk start
                        dest = grow // RPC                   # destination core
                        roff = grow % RPC
                        y_ps = psp.tile([65, 512], F32, tag="ps", name=f"yps_{b}_{h}_{qc}")
                        nkt = 4 * qc + 4                     # causal k-tiles for this chunk
                        for ki in range(nkt):
                            diag = ki // 4 == qc
                            n = 512 - (ki - 4 * qc) * 128 if diag else 512
                            qs0 = q0 + 512 - n               # within-batch q start
                            qsl = slice(b * T + qs0, b * T + q0 + 512)
                            st_ps = psp.tile([128, 512], F32, tag="ps", name=f"st_{b}_{h}_{qc}_{ki}")
                            nc.tensor.matmul(
                                st_ps[:, :n],
                                kT_sb[hp, b * T + ki * 128 : b * T + (ki + 1) * 128],
                                qT_sb[hp, qsl],
                                start=True,
                                stop=True,
                            )
                            pT = workp.tile([128, 512], BF, tag="pT")
                            nc.scalar.activation(pT[:, :n], st_ps[:, :n], Exp)
                            if diag:
                                nc.vector.tensor_tensor(
                                    pT[:, 0:128], pT[:, 0:128], mask_sb[:],
                                    mybir.AluOpType.mult,
                                )
                            nc.tensor.matmul(
                                y_ps[:, 512 - n :],
                                vp_sb[:, b * NKT + ki, vc],
                                pT[:, :n],
                                start=(ki == 0),
                                stop=(ki == nkt - 1),
                            )
                        # normalize: recip of denominator row, partition-broadcast
                        # on the (otherwise idle) gpsimd engine
                        rcp = workp.tile([1, 512], F32, tag="rcp")
                        nc.vector.reciprocal(rcp[:], y_ps[64:65, :])
                        bc_sb = workp.tile([64, 512], F32, tag="bc")
                        nc.gpsimd.partition_broadcast(bc_sb[:], rcp[:])
                        yT = workp.tile([64, 512], BF, tag="yT")
                        nc.vector.tensor_tensor(
                            yT[:], y_ps[0:64, :], bc_sb[:], mybir.AluOpType.mult
                        )
                        nc.sync.dma_start(
                            a2a_in[dest * CPC + h * 64 : dest * CPC + (h + 1) * 64,
                                   roff : roff + 512],
                            yT[:],
                        )

            # ================= Phase 3: AllToAll + output projection ====
            nc.gpsimd.collective_compute(
                "AllToAll",
                mybir.AluOpType.bypass,
                replica_groups=[list(range(NCORES))],
                ins=[a2a_in[:].opt()],
                outs=[a2a_out[:].opt()],
            )
            yTall = bigp.tile([128, 8, RPC], BF, tag="yTall")
            nc.sync.dma_start(
                yTall[:], a2a_out[:].rearrange("(ct p) r -> p ct r", p=128)
            )
            out_r = out.rearrange("(rt p) o -> p rt o", p=128)
            for rt in range(RPC // 128):
                for oc in range(2):
                    ocs = slice(oc * 512, (oc + 1) * 512)
                    o_ps = psp.tile([128, 512], F32, tag="ps", name=f"ops_{rt}_{oc}")
                    for ct in range(8):
                        nc.tensor.matmul(
                            o_ps[:],
                            yTall[:, ct, rt * 128 : (rt + 1) * 128],
                            wp_sb[:, ct, ocs],
                            start=(ct == 0),
                            stop=False,
                        )
                    # bias via ones-row rank-1 update
                    nc.tensor.matmul(
                        o_ps[:], ones_sb[:1, :], bprime_sb[:, ocs],
                        start=False, stop=True,
                    )
                    o_sb = workp.tile([128, 512], F32, tag="osb")
                    nc.vector.tensor_copy(out=o_sb[:], in_=o_ps[:])
                    nc.sync.dma_start(out_r[:, rt, ocs], o_sb[:])

    nc.finalize()
    return nc


def _prep_inputs(x, c_attn_w, c_attn_b, c_proj_w, c_proj_b):
    x = np.asarray(x, dtype=np.float32)
    c_attn_w = np.asarray(c_attn_w, dtype=np.float32)
    c_attn_b = np.asarray(c_attn_b, dtype=np.float32)
    c_proj_w = np.asarray(c_proj_w, dtype=np.float32)
    c_proj_b = np.asarray(c_proj_b, dtype=np.float32)

    xT = np.ascontiguousarray(x.reshape(ROWS, C).T).astype(BF16)
    wq, wk, wv_full = c_attn_w[:, :C], c_attn_w[:, C : 2 * C], c_attn_w[:, 2 * C :]
    bqf, bkf, bvf = c_attn_b[:C], c_attn_b[C : 2 * C], c_attn_b[2 * C :]
    wp_b = np.ascontiguousarray(c_proj_w).astype(BF16)
    bprime = (bvf @ c_proj_w + c_proj_b).reshape(1, C).astype(BF16)
    mask = np.triu(np.ones((128, 128), dtype=np.float32)).astype(BF16)

    in_maps = []
    for c in range(NCORES):
        cs = slice(c * CPC, (c + 1) * CPC)
        in_maps.append(
            {
                "xT": xT,
                "wqk": np.ascontiguousarray(
                    np.concatenate([wq[:, cs], wk[:, cs]], axis=1)
                ).astype(BF16),
                "wv": np.ascontiguousarray(wv_full[:, cs]).astype(BF16),
                "bq": np.ascontiguousarray(bqf[cs].reshape(CPC, 1) * 0.125).astype(
                    np.float32
                ),
                "bk": np.ascontiguousarray(bkf[cs].reshape(CPC, 1)).astype(np.float32),
                "wp": wp_b,
                "bprime": bprime,
                "maskd": mask,
            }
        )
    return in_maps


def kernel(x, c_attn_w, c_attn_b, c_proj_w, c_proj_b):
    from concourse.bass_utils import run_bass_kernel_spmd

    if "nc" not in _CACHE:
        _CACHE["nc"] = _build()
    nc = _CACHE["nc"]

    in_maps = _prep_inputs(x, c_attn_w, c_attn_b, c_proj_w, c_proj_b)
    res = run_bass_kernel_spmd(nc, in_maps, core_ids=list(range(NCORES)))
    full = np.concatenate([res.results[c]["out"] for c in range(NCORES)], axis=0)
    return full.reshape(B, T, C).astype(np.float32)


# revision 10
# speedup vs baseline: 1.1054x; 1.1054x over previous
"""Causal self-attention (B=4, T=2048, C=1024, H=16) on 8 trn2 NeuronCores.

Sharding: tensor-parallel over heads for QKV projection + attention
(2 heads/core), then an on-device AllToAll reshards from head-sharded to
row-sharded so each core computes the output projection (full C
contraction) for its 1024 rows. Host gather is pure concatenation.

Layout trick: attention is computed in "transposed" orientation
S^T[k, q] = (K Q^T), so softmax's reduction lands on the PSUM
accumulation path: V is augmented with a ones column, making the PV
matmul produce both y^T (rows 0..63) and the softmax denominator
(row 64) in one accumulation. No max-subtraction is needed (logits are
small: weights scaled by 0.02), and no P-transpose is needed anywhere.
"""

import sys

for _p in ("/opt/trn_rl_repo",):
    if _p not in sys.path:
        sys.path.insert(0, _p)

import numpy as np
import ml_dtypes

B, T, C, H, HS = 4, 2048, 1024, 16, 64
NCORES = 8
HPC = H // NCORES            # heads per core = 2
CPC = HPC * HS               # channels per core = 128
ROWS = B * T                 # 8192
RPC = ROWS // NCORES         # rows per core = 1024
NKT = T // 128               # k-tiles per batch = 16
NQC = T // 512               # q-chunks per batch = 4

BF16 = ml_dtypes.bfloat16

_CACHE: dict = {}


def _apply_tile_tail_patch(tile_mod):
    """This container's walrus rejects CTRL-class instructions (Drain/NoOp)
    carrying semaphore waits. Re-emit TileContext's tail waits as individual
    EventSemaphore waits and use the sem-only barrier variant."""
    import bass_rust
    from concourse.vector_clock import ScopedClock

    if getattr(tile_mod.TileContext, "_tail_patch_applied", False):
        return

    def _drain_and_barrier(self, tick_clock, wait_clock):
        collector = self.nc.sync.nop(nofuse=True, hint="tile_tail_wait")
        wait_clock.add_sem_waits(
            collector.ins, ScopedClock({None: tick_clock.global_clock})
        )
        si = collector.ins.sync_info
        waits = list(si.on_wait) if si is not None else []
        collector.ins.sync_info = None
        for w in waits:
            assert w.wait_mode == "sem-ge-imm", w
            self.nc.sync.wait_ge(
                bass_rust.SemaphoreHandle(w.ant_name, w.id), w.wait_value
            )

        self.nc.all_engine_barrier(sem_only=True)
        assert self.sems is not None
        popped = self.nc._tile_sem_poison_stack.pop()
        assert popped is self._sem_poison
        self.nc.clear_and_free_semaphores(list(self.sems.allocated().values()))
        self.nc.all_engine_barrier(sem_only=True)

    tile_mod.TileContext._drain_and_barrier = _drain_and_barrier
    tile_mod.TileContext._tail_patch_applied = True


def _build():
    import concourse.bass as bass
    import concourse.bacc as bacc
    import concourse.mybir as mybir
    import concourse.tile as tile

    dt = mybir.dt
    F32 = dt.float32
    BF = dt.bfloat16
    Exp = mybir.ActivationFunctionType.Exp
    Ident = mybir.ActivationFunctionType.Identity

    # Bacc (not plain Bass): its compile pipeline runs
    # generate_event_semaphores, which splits multi-wait sync_info into
    # EventSemaphore instructions — the walrus here accepts at most one
    # wait per instruction. It also inserts gpsimd library loads and
    # activation-table loads.
    nc = bacc.Bacc(num_devices=NCORES)

    # Inputs (per-core unless noted). xT is x transposed: [C, B*T].
    xT = nc.dram_tensor("xT", [C, ROWS], BF, kind="ExternalInput")
    wqk = nc.dram_tensor("wqk", [C, 2 * CPC], BF, kind="ExternalInput")
    wv = nc.dram_tensor("wv", [C, CPC], BF, kind="ExternalInput")
    bq = nc.dram_tensor("bq", [CPC, 1], F32, kind="ExternalInput")   # prescaled by 1/8
    bk = nc.dram_tensor("bk", [CPC, 1], F32, kind="ExternalInput")
    wp = nc.dram_tensor("wp", [C, C], BF, kind="ExternalInput")      # full c_proj_w
    bprime = nc.dram_tensor("bprime", [1, C], BF, kind="ExternalInput")
    maskd = nc.dram_tensor("maskd", [128, 128], BF, kind="ExternalInput")
    out = nc.dram_tensor("out", [RPC, C], F32, kind="ExternalOutput")

    with tile.TileContext(nc) as tc:
        with (
            tc.tile_pool(name="const", bufs=1) as constp,
            tc.tile_pool(name="big", bufs=1) as bigp,
            tc.tile_pool(name="xin", bufs=3) as xinp,
            tc.tile_pool(name="work", bufs=3) as workp,
            tc.tile_pool(name="ps", bufs=7, space="PSUM") as psp,
            tc.tile_pool(name="dram", bufs=1, space="DRAM") as dramp,
        ):
            # ---- constants ----
            wqk_sb = constp.tile([128, 8, 2 * CPC], BF, tag="wqk")
            nc.sync.dma_start(wqk_sb[:], wqk.rearrange("(ct p) o -> p ct o", p=128))
            wv_sb = constp.tile([128, 8, CPC], BF, tag="wv")
            nc.sync.dma_start(wv_sb[:], wv.rearrange("(ct p) o -> p ct o", p=128))
            wp_sb = constp.tile([128, 8, C], BF, tag="wp")
            nc.sync.dma_start(wp_sb[:], wp.rearrange("(ct p) o -> p ct o", p=128))
            bq_sb = constp.tile([CPC, 1], F32, tag="bq")
            nc.sync.dma_start(bq_sb[:], bq[:])
            bk_sb = constp.tile([CPC, 1], F32, tag="bk")
            nc.sync.dma_start(bk_sb[:], bk[:])
            bprime_sb = constp.tile([1, C], BF, tag="bprime")
            nc.sync.dma_start(bprime_sb[:], bprime[:])
            mask_sb = constp.tile([128, 128], BF, tag="mask")
            nc.sync.dma_start(mask_sb[:], maskd[:])
            ones_sb = constp.tile([1, 128], BF, tag="ones")
            nc.vector.memset(ones_sb[:], 1.0)
            onesf_sb = constp.tile([1, 64], F32, tag="onesf")
            nc.vector.memset(onesf_sb[:], 1.0)

            # ---- persistent intermediates ----
            qT_sb = bigp.tile([CPC, ROWS], BF, tag="qT")     # [2*64 ch, 8192]
            kT_sb = bigp.tile([CPC, ROWS], BF, tag="kT")
            # v' per global k-tile: [128 rows, 64 slots, 2 heads * 65] with a
            # ones column at slot offsets 64 and 129.
            vp_sb = bigp.tile([128, NKT * B, 2 * 65], BF, tag="vp")
            nc.vector.memset(vp_sb[:, :, 64:65], 1.0)
            nc.vector.memset(vp_sb[:, :, 129:130], 1.0)

            a2a_in = dramp.tile([NCORES * CPC, RPC], BF)
            a2a_out = dramp.tile([NCORES * CPC, RPC], BF)

            # ================= Phase 1: QKV projection =================
            # qT/kT: out[oc, row] = sum_c w[c, oc] * xT[c, row]
            # v:     out[row, oc] = sum_c xT[c, row] * wv[c, oc]
            xT_r = xT.rearrange("(ct p) r -> p ct r", p=128)
            for r in range(16):  # 512-row chunks
                rs = slice(r * 512, (r + 1) * 512)
                xt = xinp.tile([128, 8, 512], BF, tag="xt")
                nc.sync.dma_start(xt[:], xT_r[:, :, rs])

                q_ps = psp.tile([128, 512], F32, tag="ps", name=f"qps_{r}")
                k_ps = psp.tile([128, 512], F32, tag="ps", name=f"kps_{r}")
                # one PSUM tile (= one bank) per accumulation group: start=True
                # clears the whole bank, so groups must not share one
                v_pst = [
                    psp.tile([128, 128], F32, tag="ps", name=f"vps_{r}_{t}")
                    for t in range(4)
                ]
                for ct in range(8):
                    st, sp = (ct == 0), (ct == 7)
                    nc.tensor.matmul(
                        q_ps[:], wqk_sb[:, ct, 0:CPC], xt[:, ct, :], start=st, stop=sp
                    )
                    nc.tensor.matmul(
                        k_ps[:], wqk_sb[:, ct, CPC:], xt[:, ct, :], start=st, stop=sp
                    )
                    for t in range(4):
                        nc.tensor.matmul(
                            v_pst[t][:],
                            xt[:, ct, t * 128 : (t + 1) * 128],
                            wv_sb[:, ct, :],
                            start=st,
                            stop=sp,
                        )
                # copy-out with bias (per-partition) and 1/8 scale folded into q
                nc.scalar.activation(qT_sb[:, rs], q_ps[:], Ident, bias=bq_sb[:], scale=0.125)
                nc.scalar.activation(kT_sb[:, rs], k_ps[:], Ident, bias=bk_sb[:], scale=1.0)
                for t in range(4):
                    slot = 4 * r + t
                    nc.vector.tensor_copy(
                        out=vp_sb[:, slot, 0:64], in_=v_pst[t][:, 0:64]
                    )
                    nc.vector.tensor_copy(
                        out=vp_sb[:, slot, 65:129], in_=v_pst[t][:, 64:128]
                    )

            # ================= Phase 2: attention =================
            for b in range(B):
                for h in range(HPC):
                    hp = slice(h * 64, (h + 1) * 64)        # partition slice for this head
                    vc = slice(h * 65, (h + 1) * 65)        # v' column slice
                    for qc in range(NQC):                    # 512-wide q chunks
                        q0 = qc * 512
                        grow = b * T + q0                    # global row of chunk start
                        dest = grow // RPC                   # destination core
                        roff = grow % RPC
                        y_ps = psp.tile([65, 512], F32, tag="ps", name=f"yps_{b}_{h}_{qc}")
                        nkt = 4 * qc + 4                     # causal k-tiles for this chunk

                        def qk_exp(ki):
                            diag = ki // 4 == qc
                            n = 512 - (ki - 4 * qc) * 128 if diag else 512
                            qs0 = q0 + 512 - n               # within-batch q start
                            qsl = slice(b * T + qs0, b * T + q0 + 512)
                            st_ps = psp.tile(
                                [128, 512], F32, tag="ps", name=f"st_{b}_{h}_{qc}_{ki}"
                            )
                            nc.tensor.matmul(
                                st_ps[:, :n],
                                kT_sb[hp, b * T + ki * 128 : b * T + (ki + 1) * 128],
                                qT_sb[hp, qsl],
                                start=True,
                                stop=True,
                            )
                            pT = workp.tile([128, 512], BF, tag="pT")
                            nc.scalar.activation(pT[:, :n], st_ps[:, :n], Exp)
                            if diag:
                                nc.vector.tensor_tensor(
                                    pT[:, 0:128], pT[:, 0:128], mask_sb[:],
                                    mybir.AluOpType.mult,
                                )
                            return pT, n

                        # software pipeline: QK/exp for ki+1 issues before PV(ki)
                        # so PE never head-of-line blocks on the exp
                        pend = qk_exp(0)
                        for ki in range(nkt):
                            pT, n = pend
                            if ki + 1 < nkt:
                                pend = qk_exp(ki + 1)
                            nc.tensor.matmul(
                                y_ps[:, 512 - n :],
                                vp_sb[:, b * NKT + ki, vc],
                                pT[:, :n],
                                start=(ki == 0),
                                stop=(ki == nkt - 1),
                            )
                        # normalize: recip of denominator row, partition-broadcast
                        # on the (otherwise idle) gpsimd engine
                        den = workp.tile([1, 512], F32, tag="den")
                        nc.vector.tensor_copy(out=den[:], in_=y_ps[64:65, :])
                        rcp = workp.tile([1, 512], F32, tag="rcp")
                        nc.vector.reciprocal_approx_fast(rcp[:], den[:])
                        bc_sb = workp.tile([64, 512], F32, tag="bc")
                        nc.gpsimd.partition_broadcast(bc_sb[:], rcp[:])
                        yT = workp.tile([64, 512], BF, tag="yT")
                        nc.vector.tensor_tensor(
                            yT[:], y_ps[0:64, :], bc_sb[:], mybir.AluOpType.mult
                        )
                        nc.sync.dma_start(
                            a2a_in[dest * CPC + h * 64 : dest * CPC + (h + 1) * 64,
                                   roff : roff + 512],
                            yT[:],
                        )

            # ================= Phase 3: AllToAll + output projection ====
            nc.gpsimd.collective_compute(
                "AllToAll",
                mybir.AluOpType.bypass,
                replica_groups=[list(range(NCORES))],
                ins=[a2a_in[:].opt()],
                outs=[a2a_out[:].opt()],
            )
            yTall = bigp.tile([128, 8, RPC], BF, tag="yTall")
            nc.sync.dma_start(
                yTall[:], a2a_out[:].rearrange("(ct p) r -> p ct r", p=128)
            )
            out_r = out.rearrange("(rt p) o -> p rt o", p=128)
            for rt in range(RPC // 128):
                for oc in range(2):
                    ocs = slice(oc * 512, (oc + 1) * 512)
                    o_ps = psp.tile([128, 512], F32, tag="ps", name=f"ops_{rt}_{oc}")
                    for ct in range(8):
                        nc.tensor.matmul(
                            o_ps[:],
                            yTall[:, ct, rt * 128 : (rt + 1) * 128],
                            wp_sb[:, ct, ocs],
                            start=(ct == 0),
                            stop=False,
                        )
                    # bias via ones-row rank-1 update
                    nc.tensor.matmul(
                        o_ps[:], ones_sb[:1, :], bprime_sb[:, ocs],
                        start=False, stop=True,
                    )
                    o_sb = workp.tile([128, 512], F32, tag="osb")
                    nc.vector.tensor_copy(out=o_sb[:], in_=o_ps[:])
                    nc.sync.dma_start(out_r[:, rt, ocs], o_sb[:])

    nc.finalize()
    return nc


def _prep_inputs(x, c_attn_w, c_attn_b, c_proj_w, c_proj_b):
    x = np.asarray(x, dtype=np.float32)
    c_attn_w = np.asarray(c_attn_w, dtype=np.float32)
    c_attn_b = np.asarray(c_attn_b, dtype=np.float32)
    c_proj_w = np.asarray(c_proj_w, dtype=np.float32)
    c_proj_b = np.asarray(c_proj_b, dtype=np.float32)

    xT = np.ascontiguousarray(x.reshape(ROWS, C).T).astype(BF16)
    wq, wk, wv_full = c_attn_w[:, :C], c_attn_w[:, C : 2 * C], c_attn_w[:, 2 * C :]
    bqf, bkf, bvf = c_attn_b[:C], c_attn_b[C : 2 * C], c_attn_b[2 * C :]
    wp_b = np.ascontiguousarray(c_proj_w).astype(BF16)
    bprime = (bvf @ c_proj_w + c_proj_b).reshape(1, C).astype(BF16)
    mask = np.triu(np.ones((128, 128), dtype=np.float32)).astype(BF16)

    in_maps = []
    for c in range(NCORES):
        cs = slice(c * CPC, (c + 1) * CPC)
        in_maps.append(
            {
                "xT": xT,
                "wqk": np.ascontiguousarray(
                    np.concatenate([wq[:, cs], wk[:, cs]], axis=1)
                ).astype(BF16),
                "wv": np.ascontiguousarray(wv_full[:, cs]).astype(BF16),
                "bq": np.ascontiguousarray(bqf[cs].reshape(CPC, 1) * 0.125).astype(
                    np.float32
                ),
                "bk": np.ascontiguousarray(bkf[cs].reshape(CPC, 1)).astype(np.float32),
                "wp": wp_b,
                "bprime": bprime,
                "maskd": mask,
            }
        )
    return in_maps


def kernel(x, c_attn_w, c_attn_b, c_proj_w, c_proj_b):
    from concourse.bass_utils import run_bass_kernel_spmd

    if "nc" not in _CACHE:
        _CACHE["nc"] = _build()
    nc = _CACHE["nc"]

    in_maps = _prep_inputs(x, c_attn_w, c_attn_b, c_proj_w, c_proj_b)
    res = run_bass_kernel_spmd(nc, in_maps, core_ids=list(range(NCORES)))
    full = np.concatenate([res.results[c]["out"] for c in range(NCORES)], axis=0)
    return full.reshape(B, T, C).astype(np.float32)


# revision 14
# speedup vs baseline: 1.2029x; 1.0882x over previous
"""Causal self-attention (B=4, T=2048, C=1024, H=16) on 8 trn2 NeuronCores.

Sharding: tensor-parallel over heads for QKV projection + attention
(2 heads/core), then an on-device AllToAll reshards from head-sharded to
row-sharded so each core computes the output projection (full C
contraction) for its 1024 rows. Host gather is pure concatenation.

Layout trick: attention is computed in "transposed" orientation
S^T[k, q] = (K Q^T), so softmax's reduction lands on the PSUM
accumulation path: V is augmented with a ones column, making the PV
matmul produce both y^T (rows 0..63) and the softmax denominator
(row 64) in one accumulation. No max-subtraction is needed (logits are
small: weights scaled by 0.02), and no P-transpose is needed anywhere.
"""

import sys

for _p in ("/opt/trn_rl_repo",):
    if _p not in sys.path:
        sys.path.insert(0, _p)

import numpy as np
import ml_dtypes

B, T, C, H, HS = 4, 2048, 1024, 16, 64
NCORES = 8
HPC = H // NCORES            # heads per core = 2
CPC = HPC * HS               # channels per core = 128
ROWS = B * T                 # 8192
RPC = ROWS // NCORES         # rows per core = 1024
NKT = T // 128               # k-tiles per batch = 16
NQC = T // 512               # q-chunks per batch = 4

BF16 = ml_dtypes.bfloat16

_CACHE: dict = {}


def _apply_tile_tail_patch(tile_mod):
    """This container's walrus rejects CTRL-class instructions (Drain/NoOp)
    carrying semaphore waits. Re-emit TileContext's tail waits as individual
    EventSemaphore waits and use the sem-only barrier variant."""
    import bass_rust
    from concourse.vector_clock import ScopedClock

    if getattr(tile_mod.TileContext, "_tail_patch_applied", False):
        return

    def _drain_and_barrier(self, tick_clock, wait_clock):
        collector = self.nc.sync.nop(nofuse=True, hint="tile_tail_wait")
        wait_clock.add_sem_waits(
            collector.ins, ScopedClock({None: tick_clock.global_clock})
        )
        si = collector.ins.sync_info
        waits = list(si.on_wait) if si is not None else []
        collector.ins.sync_info = None
        for w in waits:
            assert w.wait_mode == "sem-ge-imm", w
            self.nc.sync.wait_ge(
                bass_rust.SemaphoreHandle(w.ant_name, w.id), w.wait_value
            )

        self.nc.all_engine_barrier(sem_only=True)
        assert self.sems is not None
        popped = self.nc._tile_sem_poison_stack.pop()
        assert popped is self._sem_poison
        self.nc.clear_and_free_semaphores(list(self.sems.allocated().values()))
        self.nc.all_engine_barrier(sem_only=True)

    tile_mod.TileContext._drain_and_barrier = _drain_and_barrier
    tile_mod.TileContext._tail_patch_applied = True


def _build():
    import concourse.bass as bass
    import concourse.bacc as bacc
    import concourse.mybir as mybir
    import concourse.tile as tile

    dt = mybir.dt
    F32 = dt.float32
    BF = dt.bfloat16
    Exp = mybir.ActivationFunctionType.Exp
    Ident = mybir.ActivationFunctionType.Identity

    # Bacc (not plain Bass): its compile pipeline runs
    # generate_event_semaphores, which splits multi-wait sync_info into
    # EventSemaphore instructions — the walrus here accepts at most one
    # wait per instruction. It also inserts gpsimd library loads and
    # activation-table loads.
    nc = bacc.Bacc(num_devices=NCORES)

    # Inputs (per-core unless noted). xT is x transposed: [C, B*T].
    xT = nc.dram_tensor("xT", [C, ROWS], BF, kind="ExternalInput")
    wqk = nc.dram_tensor("wqk", [C, 2 * CPC], BF, kind="ExternalInput")
    wv = nc.dram_tensor("wv", [C, CPC], BF, kind="ExternalInput")
    bq = nc.dram_tensor("bq", [CPC, 1], F32, kind="ExternalInput")   # prescaled by 1/8
    bk = nc.dram_tensor("bk", [CPC, 1], F32, kind="ExternalInput")
    wp = nc.dram_tensor("wp", [C, C], BF, kind="ExternalInput")      # full c_proj_w
    bprime = nc.dram_tensor("bprime", [1, C], BF, kind="ExternalInput")
    maskd = nc.dram_tensor("maskd", [128, 128], BF, kind="ExternalInput")
    out = nc.dram_tensor("out", [RPC, C], F32, kind="ExternalOutput")

    with tile.TileContext(nc) as tc:
        with (
            tc.tile_pool(name="const", bufs=1) as constp,
            tc.tile_pool(name="big", bufs=1) as bigp,
            tc.tile_pool(name="xin", bufs=3) as xinp,
            tc.tile_pool(name="work", bufs=3) as workp,
            tc.tile_pool(name="ps", bufs=7, space="PSUM") as psp,
            tc.tile_pool(name="dram", bufs=1, space="DRAM") as dramp,
        ):
            # ---- constants ----
            wqk_sb = constp.tile([128, 8, 2 * CPC], BF, tag="wqk")
            nc.sync.dma_start(wqk_sb[:], wqk.rearrange("(ct p) o -> p ct o", p=128))
            wv_sb = constp.tile([128, 8, CPC], BF, tag="wv")
            nc.sync.dma_start(wv_sb[:], wv.rearrange("(ct p) o -> p ct o", p=128))
            wp_sb = constp.tile([128, 8, C], BF, tag="wp")
            nc.sync.dma_start(wp_sb[:], wp.rearrange("(ct p) o -> p ct o", p=128))
            bq_sb = constp.tile([CPC, 1], F32, tag="bq")
            nc.sync.dma_start(bq_sb[:], bq[:])
            bk_sb = constp.tile([CPC, 1], F32, tag="bk")
            nc.sync.dma_start(bk_sb[:], bk[:])
            bprime_sb = constp.tile([1, C], BF, tag="bprime")
            nc.sync.dma_start(bprime_sb[:], bprime[:])
            mask_sb = constp.tile([128, 128], BF, tag="mask")
            nc.sync.dma_start(mask_sb[:], maskd[:])
            ones_sb = constp.tile([1, 128], BF, tag="ones")
            nc.vector.memset(ones_sb[:], 1.0)
            onesf_sb = constp.tile([1, 64], F32, tag="onesf")
            nc.vector.memset(onesf_sb[:], 1.0)

            # ---- persistent intermediates ----
            qT_sb = bigp.tile([CPC, ROWS], BF, tag="qT")     # [2*64 ch, 8192]
            kT_sb = bigp.tile([CPC, ROWS], BF, tag="kT")
            # v' per global k-tile: [128 rows, 64 slots, 2 heads * 65] with a
            # ones column at slot offsets 64 and 129.
            vp_sb = bigp.tile([128, NKT * B, 2 * 65], BF, tag="vp")
            nc.vector.memset(vp_sb[:, :, 64:65], 1.0)
            nc.vector.memset(vp_sb[:, :, 129:130], 1.0)

            # Two half-size AllToAll buffers: half A carries each destination
            # core's local rows 0:512 (q-chunks 0 and 2), half B rows 512:1024
            # (q-chunks 1 and 3). A fires mid-phase-2 and overlaps compute.
            a2a_in_h = [dramp.tile([NCORES * CPC, RPC // 2], BF, name=f"a2a_in{i}") for i in range(2)]
            a2a_out_h = [dramp.tile([NCORES * CPC, RPC // 2], BF, name=f"a2a_out{i}") for i in range(2)]

            # ================= Phase 1: QKV projection =================
            # qT/kT: out[oc, row] = sum_c w[c, oc] * xT[c, row]
            # v:     out[row, oc] = sum_c xT[c, row] * wv[c, oc]
            xT_r = xT.rearrange("(ct p) r -> p ct r", p=128)
            for r in range(16):  # 512-row chunks
                rs = slice(r * 512, (r + 1) * 512)
                xt = xinp.tile([128, 8, 512], BF, tag="xt")
                nc.sync.dma_start(xt[:], xT_r[:, :, rs])

                q_ps = psp.tile([128, 512], F32, tag="ps", name=f"qps_{r}")
                k_ps = psp.tile([128, 512], F32, tag="ps", name=f"kps_{r}")
                # one PSUM tile (= one bank) per accumulation group: start=True
                # clears the whole bank, so groups must not share one
                v_pst = [
                    psp.tile([128, 128], F32, tag="ps", name=f"vps_{r}_{t}")
                    for t in range(4)
                ]
                for ct in range(8):
                    st, sp = (ct == 0), (ct == 7)
                    nc.tensor.matmul(
                        q_ps[:], wqk_sb[:, ct, 0:CPC], xt[:, ct, :], start=st, stop=sp
                    )
                    nc.tensor.matmul(
                        k_ps[:], wqk_sb[:, ct, CPC:], xt[:, ct, :], start=st, stop=sp
                    )
                    for t in range(4):
                        nc.tensor.matmul(
                            v_pst[t][:],
                            xt[:, ct, t * 128 : (t + 1) * 128],
                            wv_sb[:, ct, :],
                            start=st,
                            stop=sp,
                        )
                # copy-out with bias (per-partition) and 1/8 scale folded into
                # q; on DVE to keep ACT free for phase-2 exp
                nc.vector.tensor_scalar(
                    qT_sb[:, rs], q_ps[:], bq_sb[:], 0.125,
                    mybir.AluOpType.add, mybir.AluOpType.mult,
                )
                nc.vector.tensor_scalar(
                    kT_sb[:, rs], k_ps[:], bk_sb[:], None, mybir.AluOpType.add
                )
                for t in range(4):
                    slot = 4 * r + t
                    nc.vector.tensor_copy(
                        out=vp_sb[:, slot, 0:64], in_=v_pst[t][:, 0:64]
                    )
                    nc.vector.tensor_copy(
                        out=vp_sb[:, slot, 65:129], in_=v_pst[t][:, 64:128]
                    )

            # ================= Phase 2: attention =================
            def attn_chunk(b, h, qc):
                hp = slice(h * 64, (h + 1) * 64)        # partition slice for this head
                vc = slice(h * 65, (h + 1) * 65)        # v' column slice
                q0 = qc * 512
                grow = b * T + q0                        # global row of chunk start
                dest = grow // RPC                       # destination core
                half = (grow % RPC) // 512               # which AllToAll half
                y_ps = psp.tile([65, 512], F32, tag="ps", name=f"yps_{b}_{h}_{qc}")
                nkt = 4 * qc + 4                         # causal k-tiles for this chunk

                def qk_exp(ki):
                    diag = ki // 4 == qc
                    n = 512 - (ki - 4 * qc) * 128 if diag else 512
                    qs0 = q0 + 512 - n                   # within-batch q start
                    qsl = slice(b * T + qs0, b * T + q0 + 512)
                    st_ps = psp.tile(
                        [128, 512], F32, tag="ps", name=f"st_{b}_{h}_{qc}_{ki}"
                    )
                    nc.tensor.matmul(
                        st_ps[:, :n],
                        kT_sb[hp, b * T + ki * 128 : b * T + (ki + 1) * 128],
                        qT_sb[hp, qsl],
                        start=True,
                        stop=True,
                    )
                    pT = workp.tile([128, 512], BF, tag="pT")
                    nc.scalar.activation(pT[:, :n], st_ps[:, :n], Exp)
                    if diag:
                        nc.vector.tensor_tensor(
                            pT[:, 0:128], pT[:, 0:128], mask_sb[:],
                            mybir.AluOpType.mult,
                        )
                    return pT, n

                # software pipeline: QK/exp for ki+1 issues before PV(ki)
                # so PE never head-of-line blocks on the exp
                pend = qk_exp(0)
                for ki in range(nkt):
                    pT, n = pend
                    if ki + 1 < nkt:
                        pend = qk_exp(ki + 1)
                    nc.tensor.matmul(
                        y_ps[:, 512 - n :],
                        vp_sb[:, b * NKT + ki, vc],
                        pT[:, :n],
                        start=(ki == 0),
                        stop=(ki == nkt - 1),
                    )
                # normalize: recip of denominator row, partition-broadcast
                # on the (otherwise idle) gpsimd engine
                den = workp.tile([1, 512], F32, tag="den")
                nc.vector.tensor_copy(out=den[:], in_=y_ps[64:65, :])
                rcp = workp.tile([1, 512], F32, tag="rcp")
                nc.vector.reciprocal_approx_fast(rcp[:], den[:])
                bc_sb = workp.tile([64, 512], F32, tag="bc")
                nc.gpsimd.partition_broadcast(bc_sb[:], rcp[:])
                yT = workp.tile([64, 512], BF, tag="yT")
                nc.vector.tensor_tensor(
                    yT[:], y_ps[0:64, :], bc_sb[:], mybir.AluOpType.mult
                )
                nc.sync.dma_start(
                    a2a_in_h[half][
                        dest * CPC + h * 64 : dest * CPC + (h + 1) * 64, 0:512
                    ],
                    yT[:],
                )

            def fire_a2a(half):
                nc.gpsimd.collective_compute(
                    "AllToAll",
                    mybir.AluOpType.bypass,
                    replica_groups=[list(range(NCORES))],
                    ins=[a2a_in_h[half][:].opt()],
                    outs=[a2a_out_h[half][:].opt()],
                )

            def proj_half(half):
                # output projection for my local rows [half*512, half*512+512)
                yTh = bigp.tile([128, 8, RPC // 2], BF, tag=f"yTall{half}")
                nc.sync.dma_start(
                    yTh[:], a2a_out_h[half][:].rearrange("(ct p) r -> p ct r", p=128)
                )
                out_r = out.rearrange("(rt p) o -> p rt o", p=128)
                for rt in range(4):
                    for oc in range(2):
                        ocs = slice(oc * 512, (oc + 1) * 512)
                        o_ps = psp.tile(
                            [128, 512], F32, tag="ps", name=f"ops_{half}_{rt}_{oc}"
                        )
                        for ct in range(8):
                            nc.tensor.matmul(
                                o_ps[:],
                                yTh[:, ct, rt * 128 : (rt + 1) * 128],
                                wp_sb[:, ct, ocs],
                                start=(ct == 0),
                                stop=False,
                            )
                        # bias via ones-row rank-1 update
                        nc.tensor.matmul(
                            o_ps[:], ones_sb[:1, :], bprime_sb[:, ocs],
                            start=False, stop=True,
                        )
                        o_sb = workp.tile([128, 512], F32, tag="osb")
                        nc.vector.tensor_copy(out=o_sb[:], in_=o_ps[:])
                        nc.sync.dma_start(out_r[:, half * 4 + rt, ocs], o_sb[:])

            # q-chunks 0,2 feed AllToAll half A -> fire it mid-phase so the
            # collective and the first projection half overlap the rest of
            # the attention compute
            for qc in (0, 2):
                for b in range(B):
                    for h in range(HPC):
                        attn_chunk(b, h, qc)
            fire_a2a(0)
            for qc in (1, 3):
                for b in range(B):
                    for h in range(HPC):
                        attn_chunk(b, h, qc)
            proj_half(0)
            fire_a2a(1)
            proj_half(1)

    nc.finalize()
    return nc


def _prep_inputs(x, c_attn_w, c_attn_b, c_proj_w, c_proj_b):
    x = np.asarray(x, dtype=np.float32)
    c_attn_w = np.asarray(c_attn_w, dtype=np.float32)
    c_attn_b = np.asarray(c_attn_b, dtype=np.float32)
    c_proj_w = np.asarray(c_proj_w, dtype=np.float32)
    c_proj_b = np.asarray(c_proj_b, dtype=np.float32)

    xT = np.ascontiguousarray(x.reshape(ROWS, C).T).astype(BF16)
    wq, wk, wv_full = c_attn_w[:, :C], c_attn_w[:, C : 2 * C], c_attn_w[:, 2 * C :]
    bqf, bkf, bvf = c_attn_b[:C], c_attn_b[C : 2 * C], c_attn_b[2 * C :]
    wp_b = np.ascontiguousarray(c_proj_w).astype(BF16)
    bprime = (bvf @ c_proj_w + c_proj_b).reshape(1, C).astype(BF16)
    mask = np.triu(np.ones((128, 128), dtype=np.float32)).astype(BF16)

    in_maps = []
    for c in range(NCORES):
        cs = slice(c * CPC, (c + 1) * CPC)
        in_maps.append(
            {
                "xT": xT,
                "wqk": np.ascontiguousarray(
                    np.concatenate([wq[:, cs], wk[:, cs]], axis=1)
                ).astype(BF16),
                "wv": np.ascontiguousarray(wv_full[:, cs]).astype(BF16),
                "bq": np.ascontiguousarray(bqf[cs].reshape(CPC, 1)).astype(np.float32),
                "bk": np.ascontiguousarray(bkf[cs].reshape(CPC, 1)).astype(np.float32),
                "wp": wp_b,
                "bprime": bprime,
                "maskd": mask,
            }
        )
    return in_maps


def kernel(x, c_attn_w, c_attn_b, c_proj_w, c_proj_b):
    from concourse.bass_utils import run_bass_kernel_spmd

    if "nc" not in _CACHE:
        _CACHE["nc"] = _build()
    nc = _CACHE["nc"]

    in_maps = _prep_inputs(x, c_attn_w, c_attn_b, c_proj_w, c_proj_b)
    res = run_bass_kernel_spmd(nc, in_maps, core_ids=list(range(NCORES)))
    full = np.concatenate([res.results[c]["out"] for c in range(NCORES)], axis=0)
    return full.reshape(B, T, C).astype(np.float32)


# revision 16
# speedup vs baseline: 1.2131x; 1.0084x over previous
"""Causal self-attention (B=4, T=2048, C=1024, H=16) on 8 trn2 NeuronCores.

Sharding: tensor-parallel over heads for QKV projection + attention
(2 heads/core), then an on-device AllToAll reshards from head-sharded to
row-sharded so each core computes the output projection (full C
contraction) for its 1024 rows. Host gather is pure concatenation.

Layout trick: attention is computed in "transposed" orientation
S^T[k, q] = (K Q^T), so softmax's reduction lands on the PSUM
accumulation path: V is augmented with a ones column, making the PV
matmul produce both y^T (rows 0..63) and the softmax denominator
(row 64) in one accumulation. No max-subtraction is needed (logits are
small: weights scaled by 0.02), and no P-transpose is needed anywhere.
"""

import sys

for _p in ("/opt/trn_rl_repo",):
    if _p not in sys.path:
        sys.path.insert(0, _p)

import numpy as np
import ml_dtypes

B, T, C, H, HS = 4, 2048, 1024, 16, 64
NCORES = 8
HPC = H // NCORES            # heads per core = 2
CPC = HPC * HS               # channels per core = 128
ROWS = B * T                 # 8192
RPC = ROWS // NCORES         # rows per core = 1024
NKT = T // 128               # k-tiles per batch = 16
NQC = T // 512               # q-chunks per batch = 4

BF16 = ml_dtypes.bfloat16

_CACHE: dict = {}


def _apply_tile_tail_patch(tile_mod):
    """This container's walrus rejects CTRL-class instructions (Drain/NoOp)
    carrying semaphore waits. Re-emit TileContext's tail waits as individual
    EventSemaphore waits and use the sem-only barrier variant."""
    import bass_rust
    from concourse.vector_clock import ScopedClock

    if getattr(tile_mod.TileContext, "_tail_patch_applied", False):
        return

    def _drain_and_barrier(self, tick_clock, wait_clock):
        collector = self.nc.sync.nop(nofuse=True, hint="tile_tail_wait")
        wait_clock.add_sem_waits(
            collector.ins, ScopedClock({None: tick_clock.global_clock})
        )
        si = collector.ins.sync_info
        waits = list(si.on_wait) if si is not None else []
        collector.ins.sync_info = None
        for w in waits:
            assert w.wait_mode == "sem-ge-imm", w
            self.nc.sync.wait_ge(
                bass_rust.SemaphoreHandle(w.ant_name, w.id), w.wait_value
            )

        self.nc.all_engine_barrier(sem_only=True)
        assert self.sems is not None
        popped = self.nc._tile_sem_poison_stack.pop()
        assert popped is self._sem_poison
        self.nc.clear_and_free_semaphores(list(self.sems.allocated().values()))
        self.nc.all_engine_barrier(sem_only=True)

    tile_mod.TileContext._drain_and_barrier = _drain_and_barrier
    tile_mod.TileContext._tail_patch_applied = True


def _build():
    import concourse.bass as bass
    import concourse.bacc as bacc
    import concourse.mybir as mybir
    import concourse.tile as tile

    dt = mybir.dt
    F32 = dt.float32
    BF = dt.bfloat16
    Exp = mybir.ActivationFunctionType.Exp
    Ident = mybir.ActivationFunctionType.Identity

    # Bacc (not plain Bass): its compile pipeline runs
    # generate_event_semaphores, which splits multi-wait sync_info into
    # EventSemaphore instructions — the walrus here accepts at most one
    # wait per instruction. It also inserts gpsimd library loads and
    # activation-table loads.
    nc = bacc.Bacc(num_devices=NCORES)

    # Inputs (per-core unless noted). xT is x transposed: [C, B*T].
    xT = nc.dram_tensor("xT", [C, ROWS], BF, kind="ExternalInput")
    wqk = nc.dram_tensor("wqk", [C, 2 * CPC], BF, kind="ExternalInput")
    wv = nc.dram_tensor("wv", [C, CPC], BF, kind="ExternalInput")
    bq = nc.dram_tensor("bq", [CPC, 1], F32, kind="ExternalInput")   # prescaled by 1/8
    bk = nc.dram_tensor("bk", [CPC, 1], F32, kind="ExternalInput")
    wp = nc.dram_tensor("wp", [C, C], BF, kind="ExternalInput")      # full c_proj_w
    bprime = nc.dram_tensor("bprime", [1, C], BF, kind="ExternalInput")
    maskd = nc.dram_tensor("maskd", [128, 128], BF, kind="ExternalInput")
    out = nc.dram_tensor("out", [RPC, C], F32, kind="ExternalOutput")

    with tile.TileContext(nc) as tc:
        with (
            tc.tile_pool(name="const", bufs=1) as constp,
            tc.tile_pool(name="big", bufs=1) as bigp,
            tc.tile_pool(name="xin", bufs=3) as xinp,
            tc.tile_pool(name="work", bufs=4) as workp,
            tc.tile_pool(name="ps", bufs=7, space="PSUM") as psp,
            tc.tile_pool(name="dram", bufs=1, space="DRAM") as dramp,
        ):
            # ---- constants ----
            wqk_sb = constp.tile([128, 8, 2 * CPC], BF, tag="wqk")
            nc.sync.dma_start(wqk_sb[:], wqk.rearrange("(ct p) o -> p ct o", p=128))
            wv_sb = constp.tile([128, 8, CPC], BF, tag="wv")
            nc.sync.dma_start(wv_sb[:], wv.rearrange("(ct p) o -> p ct o", p=128))
            wp_sb = constp.tile([128, 8, C], BF, tag="wp")
            nc.sync.dma_start(wp_sb[:], wp.rearrange("(ct p) o -> p ct o", p=128))
            bq_sb = constp.tile([CPC, 1], F32, tag="bq")
            nc.sync.dma_start(bq_sb[:], bq[:])
            bk_sb = constp.tile([CPC, 1], F32, tag="bk")
            nc.sync.dma_start(bk_sb[:], bk[:])
            bprime_sb = constp.tile([1, C], BF, tag="bprime")
            nc.sync.dma_start(bprime_sb[:], bprime[:])
            mask_sb = constp.tile([128, 128], BF, tag="mask")
            nc.sync.dma_start(mask_sb[:], maskd[:])
            ones_sb = constp.tile([1, 128], BF, tag="ones")
            nc.vector.memset(ones_sb[:], 1.0)
            onesf_sb = constp.tile([1, 64], F32, tag="onesf")
            nc.vector.memset(onesf_sb[:], 1.0)

            # ---- persistent intermediates ----
            qT_sb = bigp.tile([CPC, ROWS], BF, tag="qT")     # [2*64 ch, 8192]
            kT_sb = bigp.tile([CPC, ROWS], BF, tag="kT")
            # v' per global k-tile: [128 rows, 64 slots, 2 heads * 65] with a
            # ones column at slot offsets 64 and 129.
            vp_sb = bigp.tile([128, NKT * B, 2 * 65], BF, tag="vp")
            nc.vector.memset(vp_sb[:, :, 64:65], 1.0)
            nc.vector.memset(vp_sb[:, :, 129:130], 1.0)

            # Two half-size AllToAll buffers: half A carries each destination
            # core's local rows 0:512 (q-chunks 0 and 2), half B rows 512:1024
            # (q-chunks 1 and 3). A fires mid-phase-2 and overlaps compute.
            a2a_in_h = [dramp.tile([NCORES * CPC, RPC // 2], BF, name=f"a2a_in{i}") for i in range(2)]
            a2a_out_h = [dramp.tile([NCORES * CPC, RPC // 2], BF, name=f"a2a_out{i}") for i in range(2)]

            # ================= Phase 1: QKV projection =================
            # qT/kT: out[oc, row] = sum_c w[c, oc] * xT[c, row]
            # v:     out[row, oc] = sum_c xT[c, row] * wv[c, oc]
            xT_r = xT.rearrange("(ct p) r -> p ct r", p=128)
            for r in range(16):  # 512-row chunks
                rs = slice(r * 512, (r + 1) * 512)
                xt = xinp.tile([128, 8, 512], BF, tag="xt")
                nc.sync.dma_start(xt[:], xT_r[:, :, rs])

                q_ps = psp.tile([128, 512], F32, tag="ps", name=f"qps_{r}")
                k_ps = psp.tile([128, 512], F32, tag="ps", name=f"kps_{r}")
                # one PSUM tile (= one bank) per accumulation group: start=True
                # clears the whole bank, so groups must not share one
                v_pst = [
                    psp.tile([128, 128], F32, tag="ps", name=f"vps_{r}_{t}")
                    for t in range(4)
                ]
                for ct in range(8):
                    st, sp = (ct == 0), (ct == 7)
                    nc.tensor.matmul(
                        q_ps[:], wqk_sb[:, ct, 0:CPC], xt[:, ct, :], start=st, stop=sp
                    )
                    nc.tensor.matmul(
                        k_ps[:], wqk_sb[:, ct, CPC:], xt[:, ct, :], start=st, stop=sp
                    )
                    for t in range(4):
                        nc.tensor.matmul(
                            v_pst[t][:],
                            xt[:, ct, t * 128 : (t + 1) * 128],
                            wv_sb[:, ct, :],
                            start=st,
                            stop=sp,
                        )
                # copy-out with bias (per-partition) and 1/8 scale folded into
                # q; on DVE to keep ACT free for phase-2 exp
                nc.vector.tensor_scalar(
                    qT_sb[:, rs], q_ps[:], bq_sb[:], 0.125,
                    mybir.AluOpType.add, mybir.AluOpType.mult,
                )
                nc.vector.tensor_scalar(
                    kT_sb[:, rs], k_ps[:], bk_sb[:], None, mybir.AluOpType.add
                )
                for t in range(4):
                    slot = 4 * r + t
                    nc.vector.tensor_copy(
                        out=vp_sb[:, slot, 0:64], in_=v_pst[t][:, 0:64]
                    )
                    nc.vector.tensor_copy(
                        out=vp_sb[:, slot, 65:129], in_=v_pst[t][:, 64:128]
                    )

            # ================= Phase 2: attention =================
            def attn_chunk(b, h, qc):
                hp = slice(h * 64, (h + 1) * 64)        # partition slice for this head
                vc = slice(h * 65, (h + 1) * 65)        # v' column slice
                q0 = qc * 512
                grow = b * T + q0                        # global row of chunk start
                dest = grow // RPC                       # destination core
                half = (grow % RPC) // 512               # which AllToAll half
                y_ps = psp.tile([65, 512], F32, tag="ps", name=f"yps_{b}_{h}_{qc}")
                nkt = 4 * qc + 4                         # causal k-tiles for this chunk

                def qk_exp(ki):
                    diag = ki // 4 == qc
                    n = 512 - (ki - 4 * qc) * 128 if diag else 512
                    qs0 = q0 + 512 - n                   # within-batch q start
                    qsl = slice(b * T + qs0, b * T + q0 + 512)
                    st_ps = psp.tile(
                        [128, 512], F32, tag="ps", name=f"st_{b}_{h}_{qc}_{ki}"
                    )
                    nc.tensor.matmul(
                        st_ps[:, :n],
                        kT_sb[hp, b * T + ki * 128 : b * T + (ki + 1) * 128],
                        qT_sb[hp, qsl],
                        start=True,
                        stop=True,
                    )
                    pT = workp.tile([128, 512], BF, tag="pT")
                    nc.scalar.activation(pT[:, :n], st_ps[:, :n], Exp)
                    if diag:
                        nc.vector.tensor_tensor(
                            pT[:, 0:128], pT[:, 0:128], mask_sb[:],
                            mybir.AluOpType.mult,
                        )
                    return pT, n

                # software pipeline (depth 2): QK/exp for ki+2 issues before
                # PV(ki) so neither PE nor ACT waits on cross-engine latency
                pend = [qk_exp(0)]
                if nkt > 1:
                    pend.append(qk_exp(1))
                for ki in range(nkt):
                    pT, n = pend.pop(0)
                    if ki + 2 < nkt:
                        pend.append(qk_exp(ki + 2))
                    nc.tensor.matmul(
                        y_ps[:, 512 - n :],
                        vp_sb[:, b * NKT + ki, vc],
                        pT[:, :n],
                        start=(ki == 0),
                        stop=(ki == nkt - 1),
                    )
                # normalize: recip of denominator row, partition-broadcast
                # on the (otherwise idle) gpsimd engine
                den = workp.tile([1, 512], F32, tag="den")
                nc.vector.tensor_copy(out=den[:], in_=y_ps[64:65, :])
                rcp = workp.tile([1, 512], F32, tag="rcp")
                nc.vector.reciprocal_approx_fast(rcp[:], den[:])
                bc_sb = workp.tile([64, 512], F32, tag="bc")
                nc.gpsimd.partition_broadcast(bc_sb[:], rcp[:])
                yT = workp.tile([64, 512], BF, tag="yT")
                nc.vector.tensor_tensor(
                    yT[:], y_ps[0:64, :], bc_sb[:], mybir.AluOpType.mult
                )
                nc.sync.dma_start(
                    a2a_in_h[half][
                        dest * CPC + h * 64 : dest * CPC + (h + 1) * 64, 0:512
                    ],
                    yT[:],
                )

            def fire_a2a(half):
                nc.gpsimd.collective_compute(
                    "AllToAll",
                    mybir.AluOpType.bypass,
                    replica_groups=[list(range(NCORES))],
                    ins=[a2a_in_h[half][:].opt()],
                    outs=[a2a_out_h[half][:].opt()],
                )

            def proj_half(half):
                # output projection for my local rows [half*512, half*512+512)
                yTh = bigp.tile([128, 8, RPC // 2], BF, tag=f"yTall{half}")
                nc.sync.dma_start(
                    yTh[:], a2a_out_h[half][:].rearrange("(ct p) r -> p ct r", p=128)
                )
                out_r = out.rearrange("(rt p) o -> p rt o", p=128)
                for rt in range(4):
                    for oc in range(2):
                        ocs = slice(oc * 512, (oc + 1) * 512)
                        o_ps = psp.tile(
                            [128, 512], F32, tag="ps", name=f"ops_{half}_{rt}_{oc}"
                        )
                        for ct in range(8):
                            nc.tensor.matmul(
                                o_ps[:],
                                yTh[:, ct, rt * 128 : (rt + 1) * 128],
                                wp_sb[:, ct, ocs],
                                start=(ct == 0),
                                stop=False,
                            )
                        # bias via ones-row rank-1 update
                        nc.tensor.matmul(
                            o_ps[:], ones_sb[:1, :], bprime_sb[:, ocs],
                            start=False, stop=True,
                        )
                        o_sb = workp.tile([128, 512], F32, tag="osb")
                        nc.vector.tensor_copy(out=o_sb[:], in_=o_ps[:])
                        nc.sync.dma_start(out_r[:, half * 4 + rt, ocs], o_sb[:])

            # q-chunks 0,2 feed AllToAll half A -> fire it mid-phase so the
            # collective and the first projection half overlap the rest of
            # the attention compute
            for qc in (0, 2):
                for b in range(B):
                    for h in range(HPC):
                        attn_chunk(b, h, qc)
            fire_a2a(0)
            for qc in (1, 3):
                for b in range(B):
                    for h in range(HPC):
                        attn_chunk(b, h, qc)
            proj_half(0)
            fire_a2a(1)
            proj_half(1)

    nc.finalize()
    return nc


def _prep_inputs(x, c_attn_w, c_attn_b, c_proj_w, c_proj_b):
    x = np.asarray(x, dtype=np.float32)
    c_attn_w = np.asarray(c_attn_w, dtype=np.float32)
    c_attn_b = np.asarray(c_attn_b, dtype=np.float32)
    c_proj_w = np.asarray(c_proj_w, dtype=np.float32)
    c_proj_b = np.asarray(c_proj_b, dtype=np.float32)

    xT = np.ascontiguousarray(x.reshape(ROWS, C).T).astype(BF16)
    wq, wk, wv_full = c_attn_w[:, :C], c_attn_w[:, C : 2 * C], c_attn_w[:, 2 * C :]
    bqf, bkf, bvf = c_attn_b[:C], c_attn_b[C : 2 * C], c_attn_b[2 * C :]
    wp_b = np.ascontiguousarray(c_proj_w).astype(BF16)
    bprime = (bvf @ c_proj_w + c_proj_b).reshape(1, C).astype(BF16)
    mask = np.triu(np.ones((128, 128), dtype=np.float32)).astype(BF16)

    in_maps = []
    for c in range(NCORES):
        cs = slice(c * CPC, (c + 1) * CPC)
        in_maps.append(
            {
                "xT": xT,
                "wqk": np.ascontiguousarray(
                    np.concatenate([wq[:, cs], wk[:, cs]], axis=1)
                ).astype(BF16),
                "wv": np.ascontiguousarray(wv_full[:, cs]).astype(BF16),
                "bq": np.ascontiguousarray(bqf[cs].reshape(CPC, 1)).astype(np.float32),
                "bk": np.ascontiguousarray(bkf[cs].reshape(CPC, 1)).astype(np.float32),
                "wp": wp_b,
                "bprime": bprime,
                "maskd": mask,
            }
        )
    return in_maps


def kernel(x, c_attn_w, c_attn_b, c_proj_w, c_proj_b):
    from concourse.bass_utils import run_bass_kernel_spmd

    if "nc" not in _CACHE:
        _CACHE["nc"] = _build()
    nc = _CACHE["nc"]

    in_maps = _prep_inputs(x, c_attn_w, c_attn_b, c_proj_w, c_proj_b)
    res = run_bass_kernel_spmd(nc, in_maps, core_ids=list(range(NCORES)))
    full = np.concatenate([res.results[c]["out"] for c in range(NCORES)], axis=0)
    return full.reshape(B, T, C).astype(np.float32)


# revision 18
# speedup vs baseline: 1.4216x; 1.1719x over previous
"""Causal self-attention (B=4, T=2048, C=1024, H=16) on 8 trn2 NeuronCores.

Sharding: tensor-parallel over heads for QKV projection + attention
(2 heads/core), then an on-device AllToAll reshards from head-sharded to
row-sharded so each core computes the output projection (full C
contraction) for its 1024 rows. Host gather is pure concatenation.

Layout trick: attention is computed in "transposed" orientation
S^T[k, q] = (K Q^T), so softmax's reduction lands on the PSUM
accumulation path: V is augmented with a ones column, making the PV
matmul produce both y^T (rows 0..63) and the softmax denominator
(row 64) in one accumulation. No max-subtraction is needed (logits are
small: weights scaled by 0.02), and no P-transpose is needed anywhere.
"""

import sys

for _p in ("/opt/trn_rl_repo",):
    if _p not in sys.path:
        sys.path.insert(0, _p)

import numpy as np
import ml_dtypes

B, T, C, H, HS = 4, 2048, 1024, 16, 64
NCORES = 8
HPC = H // NCORES            # heads per core = 2
CPC = HPC * HS               # channels per core = 128
ROWS = B * T                 # 8192
RPC = ROWS // NCORES         # rows per core = 1024
NKT = T // 128               # k-tiles per batch = 16
NQC = T // 512               # q-chunks per batch = 4

BF16 = ml_dtypes.bfloat16

_CACHE: dict = {}


def _apply_tile_tail_patch(tile_mod):
    """This container's walrus rejects CTRL-class instructions (Drain/NoOp)
    carrying semaphore waits. Re-emit TileContext's tail waits as individual
    EventSemaphore waits and use the sem-only barrier variant."""
    import bass_rust
    from concourse.vector_clock import ScopedClock

    if getattr(tile_mod.TileContext, "_tail_patch_applied", False):
        return

    def _drain_and_barrier(self, tick_clock, wait_clock):
        collector = self.nc.sync.nop(nofuse=True, hint="tile_tail_wait")
        wait_clock.add_sem_waits(
            collector.ins, ScopedClock({None: tick_clock.global_clock})
        )
        si = collector.ins.sync_info
        waits = list(si.on_wait) if si is not None else []
        collector.ins.sync_info = None
        for w in waits:
            assert w.wait_mode == "sem-ge-imm", w
            self.nc.sync.wait_ge(
                bass_rust.SemaphoreHandle(w.ant_name, w.id), w.wait_value
            )

        self.nc.all_engine_barrier(sem_only=True)
        assert self.sems is not None
        popped = self.nc._tile_sem_poison_stack.pop()
        assert popped is self._sem_poison
        self.nc.clear_and_free_semaphores(list(self.sems.allocated().values()))
        self.nc.all_engine_barrier(sem_only=True)

    tile_mod.TileContext._drain_and_barrier = _drain_and_barrier
    tile_mod.TileContext._tail_patch_applied = True


def _build():
    import concourse.bass as bass
    import concourse.bacc as bacc
    import concourse.mybir as mybir
    import concourse.tile as tile

    dt = mybir.dt
    F32 = dt.float32
    BF = dt.bfloat16
    Exp = mybir.ActivationFunctionType.Exp
    Ident = mybir.ActivationFunctionType.Identity

    # Bacc (not plain Bass): its compile pipeline runs
    # generate_event_semaphores, which splits multi-wait sync_info into
    # EventSemaphore instructions — the walrus here accepts at most one
    # wait per instruction. It also inserts gpsimd library loads and
    # activation-table loads.
    nc = bacc.Bacc(num_devices=NCORES)

    # Inputs (per-core unless noted). xT is x transposed: [C, B*T].
    xT = nc.dram_tensor("xT", [C, ROWS], BF, kind="ExternalInput")
    wqk = nc.dram_tensor("wqk", [C, 2 * CPC], BF, kind="ExternalInput")
    wv = nc.dram_tensor("wv", [C, CPC], BF, kind="ExternalInput")
    bq = nc.dram_tensor("bq", [CPC, 1], F32, kind="ExternalInput")   # prescaled by 1/8
    bk = nc.dram_tensor("bk", [CPC, 1], F32, kind="ExternalInput")
    wp = nc.dram_tensor("wp", [C, C], BF, kind="ExternalInput")      # full c_proj_w
    bprime = nc.dram_tensor("bprime", [1, C], BF, kind="ExternalInput")
    maskd = nc.dram_tensor("maskd", [128, 128], BF, kind="ExternalInput")
    out = nc.dram_tensor("out", [RPC, C], F32, kind="ExternalOutput")

    with tile.TileContext(nc) as tc:
        with (
            tc.tile_pool(name="const", bufs=1) as constp,
            tc.tile_pool(name="big", bufs=1) as bigp,
            tc.tile_pool(name="xin", bufs=3) as xinp,
            tc.tile_pool(name="work", bufs=4) as workp,
            tc.tile_pool(name="ps", bufs=7, space="PSUM") as psp,
            tc.tile_pool(name="dram", bufs=1, space="DRAM") as dramp,
        ):
            # ---- constants ----
            wqk_sb = constp.tile([128, 8, 2 * CPC], BF, tag="wqk")
            nc.sync.dma_start(wqk_sb[:], wqk.rearrange("(ct p) o -> p ct o", p=128))
            wv_sb = constp.tile([128, 8, CPC], BF, tag="wv")
            nc.sync.dma_start(wv_sb[:], wv.rearrange("(ct p) o -> p ct o", p=128))
            wp_sb = constp.tile([128, 8, C], BF, tag="wp")
            nc.sync.dma_start(wp_sb[:], wp.rearrange("(ct p) o -> p ct o", p=128))
            bq_sb = constp.tile([CPC, 1], F32, tag="bq")
            nc.sync.dma_start(bq_sb[:], bq[:])
            bk_sb = constp.tile([CPC, 1], F32, tag="bk")
            nc.sync.dma_start(bk_sb[:], bk[:])
            bprime_sb = constp.tile([1, C], BF, tag="bprime")
            nc.sync.dma_start(bprime_sb[:], bprime[:])
            mask_sb = constp.tile([128, 128], BF, tag="mask")
            nc.sync.dma_start(mask_sb[:], maskd[:])
            ones_sb = constp.tile([1, 128], BF, tag="ones")
            nc.vector.memset(ones_sb[:], 1.0)
            onesf_sb = constp.tile([1, 64], F32, tag="onesf")
            nc.vector.memset(onesf_sb[:], 1.0)

            # ---- persistent intermediates ----
            # qT/kT: per-head slabs zero-padded from d=64 to 128 partitions so
            # attention matmuls drive the full PE array (half-array matmuls
            # keep the HAM clock-gate at 1.2 GHz; full-array runs at 2.4).
            qT_sb = bigp.tile([128, HPC, ROWS], BF, tag="qT")
            kT_sb = bigp.tile([128, HPC, ROWS], BF, tag="kT")
            nc.gpsimd.memset(qT_sb[64:128, :, :], 0.0)
            nc.gpsimd.memset(kT_sb[64:128, :, :], 0.0)
            # v' per global k-tile: [128 rows, 64 slots, 2 heads * 128]; per
            # head slot: [64 v cols | ones col | 63 zero cols] — zero-padding
            # the lhsT to 128 columns keeps the PV matmuls full-array too.
            vp_sb = bigp.tile([128, NKT * B, 2 * 128], BF, tag="vp")
            nc.gpsimd.memset(vp_sb[:, :, 65:128], 0.0)
            nc.gpsimd.memset(vp_sb[:, :, 193:256], 0.0)
            nc.vector.memset(vp_sb[:, :, 64:65], 1.0)
            nc.vector.memset(vp_sb[:, :, 192:193], 1.0)

            # Two half-size AllToAll buffers: half A carries each destination
            # core's local rows 0:512 (q-chunks 0 and 2), half B rows 512:1024
            # (q-chunks 1 and 3). A fires mid-phase-2 and overlaps compute.
            a2a_in_h = [dramp.tile([NCORES * CPC, RPC // 2], BF, name=f"a2a_in{i}") for i in range(2)]
            a2a_out_h = [dramp.tile([NCORES * CPC, RPC // 2], BF, name=f"a2a_out{i}") for i in range(2)]

            # ================= Phase 1: QKV projection =================
            # qT/kT: out[oc, row] = sum_c w[c, oc] * xT[c, row]
            # v:     out[row, oc] = sum_c xT[c, row] * wv[c, oc]
            xT_r = xT.rearrange("(ct p) r -> p ct r", p=128)
            for r in range(16):  # 512-row chunks
                rs = slice(r * 512, (r + 1) * 512)
                xt = xinp.tile([128, 8, 512], BF, tag="xt")
                nc.sync.dma_start(xt[:], xT_r[:, :, rs])

                q_ps = psp.tile([128, 512], F32, tag="ps", name=f"qps_{r}")
                k_ps = psp.tile([128, 512], F32, tag="ps", name=f"kps_{r}")
                # one PSUM tile (= one bank) per accumulation group: start=True
                # clears the whole bank, so groups must not share one
                v_pst = [
                    psp.tile([128, 128], F32, tag="ps", name=f"vps_{r}_{t}")
                    for t in range(4)
                ]
                for ct in range(8):
                    st, sp = (ct == 0), (ct == 7)
                    nc.tensor.matmul(
                        q_ps[:], wqk_sb[:, ct, 0:CPC], xt[:, ct, :], start=st, stop=sp
                    )
                    nc.tensor.matmul(
                        k_ps[:], wqk_sb[:, ct, CPC:], xt[:, ct, :], start=st, stop=sp
                    )
                    for t in range(4):
                        nc.tensor.matmul(
                            v_pst[t][:],
                            xt[:, ct, t * 128 : (t + 1) * 128],
                            wv_sb[:, ct, :],
                            start=st,
                            stop=sp,
                        )
                # copy-out with bias (per-partition) and 1/8 scale folded into
                # q; on DVE to keep ACT free for phase-2 exp
                for hh in range(HPC):
                    hs64 = slice(hh * 64, (hh + 1) * 64)
                    nc.vector.tensor_scalar(
                        qT_sb[0:64, hh, rs], q_ps[hs64, :], bq_sb[hs64, :], 0.125,
                        mybir.AluOpType.add, mybir.AluOpType.mult,
                    )
                    nc.vector.tensor_scalar(
                        kT_sb[0:64, hh, rs], k_ps[hs64, :], bk_sb[hs64, :], None,
                        mybir.AluOpType.add,
                    )
                for t in range(4):
                    slot = 4 * r + t
                    nc.vector.tensor_copy(
                        out=vp_sb[:, slot, 0:64], in_=v_pst[t][:, 0:64]
                    )
                    nc.vector.tensor_copy(
                        out=vp_sb[:, slot, 128:192], in_=v_pst[t][:, 64:128]
                    )

            # ================= Phase 2: attention =================
            def attn_chunk(b, h, qc):
                vc = slice(h * 128, h * 128 + 128)      # v' column slice (padded)
                q0 = qc * 512
                grow = b * T + q0                        # global row of chunk start
                dest = grow // RPC                       # destination core
                half = (grow % RPC) // 512               # which AllToAll half
                y_ps = psp.tile([128, 512], F32, tag="ps", name=f"yps_{b}_{h}_{qc}")
                nkt = 4 * qc + 4                         # causal k-tiles for this chunk

                def qk_exp(ki):
                    diag = ki // 4 == qc
                    n = 512 - (ki - 4 * qc) * 128 if diag else 512
                    qs0 = q0 + 512 - n                   # within-batch q start
                    qsl = slice(b * T + qs0, b * T + q0 + 512)
                    st_ps = psp.tile(
                        [128, 512], F32, tag="ps", name=f"st_{b}_{h}_{qc}_{ki}"
                    )
                    nc.tensor.matmul(
                        st_ps[:, :n],
                        kT_sb[:, h, b * T + ki * 128 : b * T + (ki + 1) * 128],
                        qT_sb[:, h, qsl],
                        start=True,
                        stop=True,
                    )
                    pT = workp.tile([128, 512], BF, tag="pT")
                    nc.scalar.activation(pT[:, :n], st_ps[:, :n], Exp)
                    if diag:
                        nc.vector.tensor_tensor(
                            pT[:, 0:128], pT[:, 0:128], mask_sb[:],
                            mybir.AluOpType.mult,
                        )
                    return pT, n

                # software pipeline (depth 2): QK/exp for ki+2 issues before
                # PV(ki) so neither PE nor ACT waits on cross-engine latency
                pend = [qk_exp(0)]
                if nkt > 1:
                    pend.append(qk_exp(1))
                for ki in range(nkt):
                    pT, n = pend.pop(0)
                    if ki + 2 < nkt:
                        pend.append(qk_exp(ki + 2))
                    nc.tensor.matmul(
                        y_ps[:, 512 - n :],
                        vp_sb[:, b * NKT + ki, vc],
                        pT[:, :n],
                        start=(ki == 0),
                        stop=(ki == nkt - 1),
                    )
                # normalize: recip of denominator row, partition-broadcast
                # on the (otherwise idle) gpsimd engine
                den = workp.tile([1, 512], F32, tag="den")
                nc.vector.tensor_copy(out=den[:], in_=y_ps[64:65, :])
                rcp = workp.tile([1, 512], F32, tag="rcp")
                nc.vector.reciprocal_approx_fast(rcp[:], den[:])
                bc_sb = workp.tile([64, 512], F32, tag="bc")
                nc.gpsimd.partition_broadcast(bc_sb[:], rcp[:])
                yT = workp.tile([64, 512], BF, tag="yT")
                nc.vector.tensor_tensor(
                    yT[:], y_ps[0:64, :], bc_sb[:], mybir.AluOpType.mult
                )
                nc.sync.dma_start(
                    a2a_in_h[half][
                        dest * CPC + h * 64 : dest * CPC + (h + 1) * 64, 0:512
                    ],
                    yT[:],
                )

            def fire_a2a(half):
                nc.gpsimd.collective_compute(
                    "AllToAll",
                    mybir.AluOpType.bypass,
                    replica_groups=[list(range(NCORES))],
                    ins=[a2a_in_h[half][:].opt()],
                    outs=[a2a_out_h[half][:].opt()],
                )

            def proj_half(half):
                # output projection for my local rows [half*512, half*512+512)
                yTh = bigp.tile([128, 8, RPC // 2], BF, tag=f"yTall{half}")
                nc.sync.dma_start(
                    yTh[:], a2a_out_h[half][:].rearrange("(ct p) r -> p ct r", p=128)
                )
                out_r = out.rearrange("(rt p) o -> p rt o", p=128)
                for rt in range(4):
                    for oc in range(2):
                        ocs = slice(oc * 512, (oc + 1) * 512)
                        o_ps = psp.tile(
                            [128, 512], F32, tag="ps", name=f"ops_{half}_{rt}_{oc}"
                        )
                        for ct in range(8):
                            nc.tensor.matmul(
                                o_ps[:],
                                yTh[:, ct, rt * 128 : (rt + 1) * 128],
                                wp_sb[:, ct, ocs],
                                start=(ct == 0),
                                stop=False,
                            )
                        # bias via ones-row rank-1 update
                        nc.tensor.matmul(
                            o_ps[:], ones_sb[:1, :], bprime_sb[:, ocs],
                            start=False, stop=True,
                        )
                        o_sb = workp.tile([128, 512], F32, tag="osb")
                        nc.vector.tensor_copy(out=o_sb[:], in_=o_ps[:])
                        nc.sync.dma_start(out_r[:, half * 4 + rt, ocs], o_sb[:])

            # q-chunks 0,2 feed AllToAll half A -> fire it mid-phase so the
            # collective and the first projection half overlap the rest of
            # the attention compute
            for qc in (0, 2):
                for b in range(B):
                    for h in range(HPC):
                        attn_chunk(b, h, qc)
            fire_a2a(0)
            for qc in (1, 3):
                for b in range(B):
                    for h in range(HPC):
                        attn_chunk(b, h, qc)
            proj_half(0)
            fire_a2a(1)
            proj_half(1)

    nc.finalize()
    return nc


def _prep_inputs(x, c_attn_w, c_attn_b, c_proj_w, c_proj_b):
    x = np.asarray(x, dtype=np.float32)
    c_attn_w = np.asarray(c_attn_w, dtype=np.float32)
    c_attn_b = np.asarray(c_attn_b, dtype=np.float32)
    c_proj_w = np.asarray(c_proj_w, dtype=np.float32)
    c_proj_b = np.asarray(c_proj_b, dtype=np.float32)

    xT = np.ascontiguousarray(x.reshape(ROWS, C).T).astype(BF16)
    wq, wk, wv_full = c_attn_w[:, :C], c_attn_w[:, C : 2 * C], c_attn_w[:, 2 * C :]
    bqf, bkf, bvf = c_attn_b[:C], c_attn_b[C : 2 * C], c_attn_b[2 * C :]
    wp_b = np.ascontiguousarray(c_proj_w).astype(BF16)
    bprime = (bvf @ c_proj_w + c_proj_b).reshape(1, C).astype(BF16)
    mask = np.triu(np.ones((128, 128), dtype=np.float32)).astype(BF16)

    in_maps = []
    for c in range(NCORES):
        cs = slice(c * CPC, (c + 1) * CPC)
        in_maps.append(
            {
                "xT": xT,
                "wqk": np.ascontiguousarray(
                    np.concatenate([wq[:, cs], wk[:, cs]], axis=1)
                ).astype(BF16),
                "wv": np.ascontiguousarray(wv_full[:, cs]).astype(BF16),
                "bq": np.ascontiguousarray(bqf[cs].reshape(CPC, 1)).astype(np.float32),
                "bk": np.ascontiguousarray(bkf[cs].reshape(CPC, 1)).astype(np.float32),
                "wp": wp_b,
                "bprime": bprime,
                "maskd": mask,
            }
        )
    return in_maps


def kernel(x, c_attn_w, c_attn_b, c_proj_w, c_proj_b):
    from concourse.bass_utils import run_bass_kernel_spmd

    if "nc" not in _CACHE:
        _CACHE["nc"] = _build()
    nc = _CACHE["nc"]

    in_maps = _prep_inputs(x, c_attn_w, c_attn_b, c_proj_w, c_proj_b)
    res = run_bass_kernel_spmd(nc, in_maps, core_ids=list(range(NCORES)))
    full = np.concatenate([res.results[c]["out"] for c in range(NCORES)], axis=0)
    return full.reshape(B, T, C).astype(np.float32)
